# revision 13
# baseline (speedup 1.0000x reference)
"""HGT layer kernel for nn_HGTLayer_53188874994368 on 8 TRN2 NeuronCores.

Strategy (dst-partitioned edge parallel):
  - Destination nodes are partitioned into 8 slabs of 6400 rows (N padded
    to 51200).  Edges of each etype are sorted by destination and routed
    to the core owning the destination slab, so segment-softmax and
    scatter-sum are fully local per core (no cross-core reduction).
  - Node features are sharded across cores (transposed bf16 layout) and
    AllGathered on-device over NeuronLink; K/V projection tables for all
    nodes (projection weights pre-fused with the per-etype head
    transforms and mu/sqrt(dk) scaling) are computed per core on TensorE.
  - Per 128-edge tile: indirect-DMA gathers of K||V (merged table) and Q
    rows, per-head dot scores on VectorE, exp on ScalarE, one-hot segment
    matmul on TensorE, and an indirect-DMA scatter of per-segment partial
    sums.  Tiles alternate between two accumulator slabs (a segment spans
    at most 2 consecutive tiles, so each slab sees each row at most
    once); the final pass combines them, divides by the softmax
    denominator and applies the skip-gated output projection + LayerNorm
    on-device.
  - All per-core inputs travel as ONE packed bf16 blob (the axon tunnel
    charges per-array overhead), outputs as one bf16 tensor; the PJRT
    executable is jitted and warmed at import time with device-side
    donation buffers so a kernel() call pays only input upload, execute
    and output download.

Host side only sorts edges / packs index tiles (u16) and re-assembles
the output; all FLOP/bandwidth-heavy work runs on the NeuronCores.
"""

import math

import numpy as np

H = 4
DK = 32
D = 128
P = 128
N = 50000
NPAD = 51200           # 8 * 6400
NSLAB = 6400           # dst rows per core
SLAB_ROWS = NSLAB + P  # + dummy rows for padding segments
FCOLS = D + H          # 128 message cols + 4 denominator cols
LN_EPS = 1e-5
NCORES = 8
_EXPECTED_T = 608      # edge tiles per etype for the reference problem size

# ---- packed input blob layout (bf16 columns, [128, BLOBW]) ----
_SH = NSLAB + P        # feature shard cols incl. slack


def _layout(T):
    EPK = 4 * T + 32
    off = {}
    o = 0
    for k, w in (("FU", _SH), ("FI", _SH), ("EPU", EPK), ("EPI", EPK),
                 ("KVW_UI", 2 * D), ("KVW_IU", 2 * D), ("QW_UI", D),
                 ("QW_IU", D), ("WA_U", D), ("WA_I", D), ("IFAC_U", D),
                 ("IFAC_I", D), ("VEC", 4 * D)):
        off[k] = o
        o += w
    return off, o, EPK


_PROGRAM_CACHE = {}
_RUNNER_CACHE = {}


def _ensure_imports():
    import sys
    for p in ("/opt/trn_rl_repo", "/root/.axon_site/_ro/trn_rl_repo"):
        if p not in sys.path:
            sys.path.append(p)


# ----------------------------------------------------------------------------
# Device program
# ----------------------------------------------------------------------------
def _build_program(T):
    """Build the (core-uniform) Bass program for T edge-tiles per etype."""
    import concourse.bacc as bacc
    import concourse.bass as bass
    import concourse.mybir as mybir
    import concourse.tile as tile
    from concourse.bass import IndirectOffsetOnAxis
    from concourse.masks import make_identity

    bf16 = mybir.dt.bfloat16
    f32 = mybir.dt.float32
    i32 = mybir.dt.int32
    u16 = mybir.dt.uint16

    OFF, BLOBW, EPK = _layout(T)

    nc = bacc.Bacc("TRN2", target_bir_lowering=False, debug=False,
                   num_devices=NCORES, disable_frame_to_traceback=True)

    blob = nc.dram_tensor("blob", [P, BLOBW], bf16, kind="ExternalInput")
    outb = nc.dram_tensor("outb", [2 * SLAB_ROWS, D], bf16,
                          kind="ExternalOutput")

    def bsl(key, w, r0=0, rows=P):
        return blob[r0:r0 + rows, OFF[key]:OFF[key] + w]

    # ---- internal DRAM ----
    ccin_u = nc.dram_tensor("ccin_u", [P, _SH], bf16, kind="Internal")
    ccin_i = nc.dram_tensor("ccin_i", [P, _SH], bf16, kind="Internal")
    ccout_u = nc.dram_tensor("ccout_u", [NCORES, P, _SH], bf16,
                             kind="Internal", addr_space="Shared")
    ccout_i = nc.dram_tensor("ccout_i", [NCORES, P, _SH], bf16,
                             kind="Internal", addr_space="Shared")
    KV_ui = nc.dram_tensor("KV_ui", [NPAD + P, 2 * D], bf16, kind="Internal")
    KV_iu = nc.dram_tensor("KV_iu", [NPAD + P, 2 * D], bf16, kind="Internal")
    Q_ui = nc.dram_tensor("Q_ui", [SLAB_ROWS, D], bf16, kind="Internal")
    Q_iu = nc.dram_tensor("Q_iu", [SLAB_ROWS, D], bf16, kind="Internal")
    slabs = {}
    for et in ("ui", "iu"):
        for col in ("A", "B"):
            slabs[et, col] = nc.dram_tensor(
                f"slab{col}_{et}", [SLAB_ROWS, FCOLS], f32, kind="Internal")

    with tile.TileContext(nc) as tc:
        # ---------------- phase 0: allgather features ----------------
        with tc.tile_pool(name="p0", bufs=2) as p0:
            for key, cci in (("FU", ccin_u), ("FI", ccin_i)):
                stg = p0.tile([P, _SH], bf16, tag="stg")
                nc.sync.dma_start(out=stg[:], in_=bsl(key, _SH))
                nc.sync.dma_start(out=cci[:, :], in_=stg[:])
        nc.gpsimd.collective_compute(
            "AllGather", mybir.AluOpType.bypass,
            ins=[ccin_u[:, :]], outs=[ccout_u[:, :, :]],
            replica_groups=[list(range(NCORES))])
        nc.gpsimd.collective_compute(
            "AllGather", mybir.AluOpType.bypass,
            ins=[ccin_i[:, :]], outs=[ccout_i[:, :, :]],
            replica_groups=[list(range(NCORES))])

        # ---------------- constants + weights in SBUF ----------------
        with tc.tile_pool(name="const", bufs=1) as cpool:
            zt = cpool.tile([P, 4096], f32)
            nc.vector.memset(zt[:], 0.0)
            for et in ("ui", "iu"):
                for col in ("A", "B"):
                    flat = slabs[et, col].ap().rearrange("a b -> (a b)")
                    percol = SLAB_ROWS * FCOLS // P
                    v = flat.rearrange("(p x) -> p x", p=P)
                    for c0 in range(0, percol, 4096):
                        w = min(4096, percol - c0)
                        nc.sync.dma_start(out=v[:, c0:c0 + w], in_=zt[:, :w])

            iota_i = cpool.tile([P, P], i32)
            nc.gpsimd.iota(iota_i[:], pattern=[[1, P]], base=0,
                           channel_multiplier=0)
            iota_f = cpool.tile([P, P], f32)
            nc.vector.tensor_copy(out=iota_f[:], in_=iota_i[:])
            ident_f = cpool.tile([P, P], f32)
            make_identity(nc, ident_f[:])
            ones1 = cpool.tile([1, P], bf16)
            nc.vector.memset(ones1[:], 1.0)
            ones1f = cpool.tile([1, P], f32)
            nc.vector.memset(ones1f[:], 1.0)
            epsc = cpool.tile([P, 1], f32)
            nc.vector.memset(epsc[:], LN_EPS)

            def load_mat(key, w, name):
                t = cpool.tile([P, w], bf16, name=name, tag=name)
                nc.sync.dma_start(out=t[:], in_=bsl(key, w))
                return t

            def load_vec(row, w, name):
                t = cpool.tile([1, w], bf16, name=name, tag=name)
                nc.sync.dma_start(out=t[:], in_=bsl("VEC", w, r0=row, rows=1))
                return t

            kvw_s = {"ui": load_mat("KVW_UI", 2 * D, "kvw_ui_s"),
                     "iu": load_mat("KVW_IU", 2 * D, "kvw_iu_s")}
            qw_s = {"ui": load_mat("QW_UI", D, "qw_ui_s"),
                    "iu": load_mat("QW_IU", D, "qw_iu_s")}
            wa_s = {"u": load_mat("WA_U", D, "wa_u_s"),
                    "i": load_mat("WA_I", D, "wa_i_s")}
            ifac_s = {"u": load_mat("IFAC_U", D, "ifac_u_s"),
                      "i": load_mat("IFAC_I", D, "ifac_i_s")}
            kvb_s = {"ui": load_vec(0, 2 * D, "kvb_ui_s"),
                     "iu": load_vec(1, 2 * D, "kvb_iu_s")}
            qb_s = {"ui": load_vec(2, D, "qb_ui_s"),
                    "iu": load_vec(3, D, "qb_iu_s")}
            wab_s = {"u": load_vec(4, D, "wab_u_s"),
                     "i": load_vec(5, D, "wab_i_s")}
            lnraw = {"u": load_vec(6, 4 * D, "lnraw_u"),
                     "i": load_vec(7, 4 * D, "lnraw_i")}

            # row-broadcast LN gamma||beta via K=1 matmul (f32)
            lnb = {}
            with tc.tile_pool(name="pgb", bufs=2, space="PSUM") as pgb:
                for nt in ("u", "i"):
                    ps = pgb.tile([P, 2 * D], f32, space="PSUM", tag="ps",
                                  name=f"psln_{nt}")
                    nc.tensor.matmul(out=ps[:], lhsT=ones1f[:],
                                     rhs=lnraw[nt][:].bitcast(f32),
                                     start=True, stop=True)
                    gb = cpool.tile([P, 2 * D], f32, name=f"lnbc_{nt}",
                                    tag=f"lnbc_{nt}")
                    nc.vector.tensor_copy(out=gb[:], in_=ps[:])
                    lnb[nt] = gb

            # epacks to SBUF (bf16-viewed u16 -> i32 once)
            ep_sb = {}
            for et, key in (("ui", "EPU"), ("iu", "EPI")):
                raw = cpool.tile([P, EPK], bf16, name=f"epraw_{et}",
                                 tag=f"epraw_{et}")
                nc.sync.dma_start(out=raw[:], in_=bsl(key, EPK))
                conv = cpool.tile([P, EPK], i32, name=f"ep_{et}",
                                  tag=f"ep_{et}")
                nc.vector.tensor_copy(out=conv[:], in_=raw[:].bitcast(u16))
                ep_sb[et] = conv

            # ---------------- phase 1: projection tables ----------------
            NT_B = NSLAB // P  # 50 tiles per core block
            with tc.tile_pool(name="tb", bufs=3) as tb, \
                 tc.tile_pool(name="tbp", bufs=1, space="PSUM") as tbp:
                for c in range(NCORES):
                    with tc.For_i(0, NT_B, 2) as j:
                        for uu in range(2):
                            col = (j + uu) * P
                            fu_t = tb.tile([P, P], bf16, tag=f"fu_t{uu}",
                                           name=f"fu_t{uu}")
                            nc.sync.dma_start(
                                out=fu_t[:],
                                in_=ccout_u[c, :, bass.ds(col, P)])
                            fi_t = tb.tile([P, P], bf16, tag=f"fi_t{uu}",
                                           name=f"fi_t{uu}")
                            nc.sync.dma_start(
                                out=fi_t[:],
                                in_=ccout_i[c, :, bass.ds(col, P)])
                            for et, ft in (("ui", fu_t), ("iu", fi_t)):
                                KVt = KV_ui if et == "ui" else KV_iu
                                ps = tbp.tile([P, 2 * D], f32, space="PSUM",
                                              tag=f"pkv{uu}_{et}",
                                              name=f"pkv{uu}_{et}")
                                nc.tensor.matmul(out=ps[:], lhsT=ft[:],
                                                 rhs=kvw_s[et][:], start=True,
                                                 stop=False)
                                nc.tensor.matmul(out=ps[:], lhsT=ones1[:],
                                                 rhs=kvb_s[et][:], start=False,
                                                 stop=True)
                                sb = tb.tile([P, 2 * D], bf16,
                                             tag=f"kvsb{uu}_{et}",
                                             name=f"kvsb{uu}_{et}")
                                nc.vector.tensor_copy(out=sb[:], in_=ps[:])
                                nc.sync.dma_start(
                                    out=KVt[c * NSLAB:(c + 1) * NSLAB + P, :]
                                        .rearrange("(t p) d -> t p d", p=P)
                                        [bass.ds(j + uu, 1), :, :],
                                    in_=sb[:])

                # Q tables for own slab (from own shard region of blob)
                with tc.For_i(0, NT_B, 2) as j:
                    for uu in range(2):
                        col = (j + uu) * P
                        fu_t = tb.tile([P, P], bf16, tag=f"qfu_t{uu}",
                                       name=f"qfu_t{uu}")
                        nc.sync.dma_start(
                            out=fu_t[:],
                            in_=blob[:, OFF["FU"]:OFF["FU"] + _SH]
                                [:, bass.ds(col, P)])
                        fi_t = tb.tile([P, P], bf16, tag=f"qfi_t{uu}",
                                       name=f"qfi_t{uu}")
                        nc.sync.dma_start(
                            out=fi_t[:],
                            in_=blob[:, OFF["FI"]:OFF["FI"] + _SH]
                                [:, bass.ds(col, P)])
                        # Q_ui: dst = item -> item feats; Q_iu: dst = user
                        for et, ft in (("ui", fi_t), ("iu", fu_t)):
                            Qt = Q_ui if et == "ui" else Q_iu
                            ps = tbp.tile([P, D], f32, space="PSUM",
                                          tag=f"pq{uu}_{et}",
                                          name=f"pq{uu}_{et}")
                            nc.tensor.matmul(out=ps[:], lhsT=ft[:],
                                             rhs=qw_s[et][:], start=True,
                                             stop=False)
                            nc.tensor.matmul(out=ps[:], lhsT=ones1[:],
                                             rhs=qb_s[et][:], start=False,
                                             stop=True)
                            sb = tb.tile([P, D], bf16, tag=f"qsb{uu}_{et}",
                                         name=f"qsb{uu}_{et}")
                            nc.vector.tensor_copy(out=sb[:], in_=ps[:])
                            nc.sync.dma_start(
                                out=Qt[0:NSLAB + P, :]
                                    .rearrange("(t p) d -> t p d", p=P)
                                    [bass.ds(j + uu, 1), :, :],
                                in_=sb[:])

            # ---------------- phase 2: edge aggregation ----------------
            UN = 8
            for et in ("ui", "iu"):
                KVt = KV_ui if et == "ui" else KV_iu
                Qt = Q_ui if et == "ui" else Q_iu
                eps = ep_sb[et]
                slA, slB = slabs[et, "A"], slabs[et, "B"]
                with tc.tile_pool(name=f"ew_{et}", bufs=2) as wp, \
                     tc.tile_pool(name=f"epp_{et}", bufs=1, space="PSUM") as pp:
                    stages = [cpool.tile([P, 4], i32, tag=f"stg_{et}{u}",
                                         name=f"stg_{et}{u}")
                              for u in range(UN)]
                    with tc.For_i(0, T, UN) as it:
                        for u in range(UN):
                            st = stages[u]
                            nc.vector.tensor_copy(
                                out=st[:],
                                in_=eps[:, bass.ds((it + u) * 4, 4)])
                            segf = wp.tile([P, 1], f32, tag=f"segf{u}",
                                           name=f"segf{u}")
                            nc.vector.tensor_copy(out=segf[:], in_=st[:, 3:4])
                            kvg = wp.tile([P, 2 * D], bf16, tag=f"kvg{u}",
                                          name=f"kvg{u}")
                            nc.gpsimd.indirect_dma_start(
                                out=kvg[:], out_offset=None, in_=KVt[:, :],
                                in_offset=IndirectOffsetOnAxis(
                                    ap=st[:, 0:1], axis=0))
                            qg = wp.tile([P, D], bf16, tag=f"qg{u}",
                                         name=f"qg{u}")
                            nc.gpsimd.indirect_dma_start(
                                out=qg[:], out_offset=None, in_=Qt[:, :],
                                in_offset=IndirectOffsetOnAxis(
                                    ap=st[:, 1:2], axis=0))
                            prod = wp.tile([P, D], f32, tag=f"prod{u}",
                                           name=f"prod{u}")
                            nc.vector.tensor_mul(out=prod[:], in0=kvg[:, 0:D],
                                                 in1=qg[:])
                            s = wp.tile([P, H], f32, tag=f"s{u}", name=f"s{u}")
                            nc.vector.tensor_reduce(
                                out=s[:],
                                in_=prod[:].rearrange("p (h d) -> p h d", h=H),
                                op=mybir.AluOpType.add,
                                axis=mybir.AxisListType.X)
                            w = wp.tile([P, H], f32, tag=f"w{u}", name=f"w{u}")
                            nc.scalar.activation(
                                out=w[:], in_=s[:],
                                func=mybir.ActivationFunctionType.Exp)
                            m = wp.tile([P, FCOLS], bf16, tag=f"m{u}",
                                        name=f"m{u}")
                            for h in range(H):
                                nc.vector.tensor_mul(
                                    out=m[:, h * DK:(h + 1) * DK],
                                    in0=kvg[:, D + h * DK:D + (h + 1) * DK],
                                    in1=w[:, h:h + 1].to_broadcast([P, DK]))
                            nc.vector.tensor_copy(out=m[:, D:FCOLS], in_=w[:])
                            S = wp.tile([P, P], bf16, tag=f"S{u}",
                                        name=f"S{u}")
                            nc.vector.tensor_tensor(
                                out=S[:],
                                in0=segf[:].to_broadcast([P, P]),
                                in1=iota_f[:], op=mybir.AluOpType.is_equal)
                            acc = pp.tile([P, FCOLS], f32, space="PSUM",
                                          tag=f"acc{u}", name=f"acc{u}")
                            nc.tensor.matmul(out=acc[:], lhsT=S[:], rhs=m[:],
                                             start=True, stop=True)
                            accs = wp.tile([P, FCOLS], f32, tag=f"accs{u}",
                                           name=f"accs{u}")
                            nc.vector.tensor_copy(out=accs[:], in_=acc[:])
                            slab = slA if u % 2 == 0 else slB
                            nc.gpsimd.indirect_dma_start(
                                out=slab[:, :],
                                out_offset=IndirectOffsetOnAxis(
                                    ap=st[:, 2:3], axis=0),
                                in_=accs[:], in_offset=None)

            # -------------- phase 3: combine + node_out + LN --------------
            for nt in ("u", "i"):
                # h_user comes from etype iu, h_item from ui
                et = "iu" if nt == "u" else "ui"
                slA, slB = slabs[et, "A"], slabs[et, "B"]
                fkey = "FU" if nt == "u" else "FI"
                nt_off = 0 if nt == "u" else SLAB_ROWS
                gbc = lnb[nt]
                with tc.tile_pool(name=f"no_{nt}", bufs=3) as op, \
                     tc.tile_pool(name=f"nop_{nt}", bufs=2, space="PSUM") as pq:
                    with tc.For_i(0, NSLAB // P, 2) as j:
                        for uu in range(2):
                            a = op.tile([P, FCOLS], f32, tag=f"fa{uu}",
                                        name=f"fa{uu}")
                            nc.sync.dma_start(
                                out=a[:],
                                in_=slA[0:NSLAB + P, :]
                                    .rearrange("(t p) d -> t p d", p=P)
                                    [bass.ds(j + uu, 1), :, :])
                            b = op.tile([P, FCOLS], f32, tag=f"fb{uu}",
                                        name=f"fb{uu}")
                            nc.sync.dma_start(
                                out=b[:],
                                in_=slB[0:NSLAB + P, :]
                                    .rearrange("(t p) d -> t p d", p=P)
                                    [bass.ds(j + uu, 1), :, :])
                            nc.vector.tensor_add(out=a[:], in0=a[:], in1=b[:])
                            den = op.tile([P, H], f32, tag=f"den{uu}",
                                          name=f"den{uu}")
                            nc.vector.tensor_scalar_add(den[:], a[:, D:FCOLS],
                                                        1e-30)
                            rec = op.tile([P, H], f32, tag=f"rec{uu}",
                                          name=f"rec{uu}")
                            nc.vector.reciprocal(out=rec[:], in_=den[:])
                            hb = op.tile([P, D], f32, tag=f"hb{uu}",
                                         name=f"hb{uu}")
                            for h in range(H):
                                nc.vector.tensor_mul(
                                    out=hb[:, h * DK:(h + 1) * DK],
                                    in0=a[:, h * DK:(h + 1) * DK],
                                    in1=rec[:, h:h + 1].to_broadcast([P, DK]))
                            hT_ps = pq.tile([P, D], f32, space="PSUM",
                                            tag=f"hT{uu}", name=f"hT{uu}")
                            nc.tensor.transpose(out=hT_ps[:], in_=hb[:],
                                                identity=ident_f[:])
                            hT = op.tile([P, D], bf16, tag=f"hTs{uu}",
                                         name=f"hTs{uu}")
                            nc.vector.tensor_copy(out=hT[:], in_=hT_ps[:])
                            fsh_t = op.tile([P, P], bf16, tag=f"fsh{uu}",
                                            name=f"fsh{uu}")
                            nc.sync.dma_start(
                                out=fsh_t[:],
                                in_=blob[:, OFF[fkey]:OFF[fkey] + _SH]
                                    [:, bass.ds((j + uu) * P, P)])
                            ops_ = pq.tile([P, D], f32, space="PSUM",
                                           tag=f"ops{uu}", name=f"ops{uu}")
                            nc.tensor.matmul(out=ops_[:], lhsT=hT[:],
                                             rhs=wa_s[nt][:], start=True,
                                             stop=False)
                            nc.tensor.matmul(out=ops_[:], lhsT=ones1[:],
                                             rhs=wab_s[nt][:], start=False,
                                             stop=False)
                            nc.tensor.matmul(out=ops_[:], lhsT=fsh_t[:],
                                             rhs=ifac_s[nt][:], start=False,
                                             stop=True)
                            o = op.tile([P, D], f32, tag=f"o{uu}",
                                        name=f"o{uu}")
                            nc.vector.tensor_copy(out=o[:], in_=ops_[:])
                            mean = op.tile([P, 1], f32, tag=f"mean{uu}",
                                           name=f"mean{uu}")
                            nc.vector.tensor_reduce(
                                out=mean[:], in_=o[:],
                                op=mybir.AluOpType.add,
                                axis=mybir.AxisListType.X)
                            nc.vector.tensor_scalar_mul(mean[:], mean[:],
                                                        1.0 / D)
                            oc = op.tile([P, D], f32, tag=f"oc{uu}",
                                         name=f"oc{uu}")
                            nc.vector.tensor_tensor(
                                out=oc[:], in0=o[:],
                                in1=mean[:].to_broadcast([P, D]),
                                op=mybir.AluOpType.subtract)
                            sq = op.tile([P, D], f32, tag=f"sq{uu}",
                                         name=f"sq{uu}")
                            var = op.tile([P, 1], f32, tag=f"var{uu}",
                                          name=f"var{uu}")
                            nc.vector.tensor_mul(out=sq[:], in0=oc[:],
                                                 in1=oc[:])
                            nc.vector.tensor_reduce(
                                out=var[:], in_=sq[:],
                                op=mybir.AluOpType.add,
                                axis=mybir.AxisListType.X)
                            nc.vector.tensor_scalar_mul(var[:], var[:],
                                                        1.0 / D)
                            std = op.tile([P, 1], f32, tag=f"std{uu}",
                                          name=f"std{uu}")
                            nc.scalar.activation(
                                out=std[:], in_=var[:],
                                func=mybir.ActivationFunctionType.Sqrt,
                                bias=epsc[:])
                            rstd = op.tile([P, 1], f32, tag=f"rstd{uu}",
                                           name=f"rstd{uu}")
                            nc.vector.reciprocal(out=rstd[:], in_=std[:])
                            z = op.tile([P, D], f32, tag=f"z{uu}",
                                        name=f"z{uu}")
                            nc.vector.tensor_mul(
                                out=z[:], in0=oc[:],
                                in1=rstd[:].to_broadcast([P, D]))
                            zg = op.tile([P, D], f32, tag=f"zg{uu}",
                                         name=f"zg{uu}")
                            nc.vector.tensor_mul(out=zg[:], in0=z[:],
                                                 in1=gbc[:, 0:D])
                            ob = op.tile([P, D], bf16, tag=f"ob{uu}",
                                         name=f"ob{uu}")
                            nc.vector.tensor_add(out=ob[:], in0=zg[:],
                                                 in1=gbc[:, D:2 * D])
                            nc.sync.dma_start(
                                out=outb[nt_off:nt_off + SLAB_ROWS, :]
                                    .rearrange("(t p) d -> t p d", p=P)
                                    [bass.ds(j + uu, 1), :, :],
                                in_=ob[:])

    nc.compile()
    return nc


# ----------------------------------------------------------------------------
# Custom PJRT runner (jitted once; device-side donation zeros)
# ----------------------------------------------------------------------------
def _make_runner(nc):
    import jax
    import jax.numpy as jnp
    from jax.experimental.shard_map import shard_map
    from jax.sharding import Mesh, NamedSharding, PartitionSpec
    import concourse.mybir as mybir
    from concourse import bass2jax

    bass2jax.install_neuronx_cc_hook()

    partition_name = (nc.partition_id_tensor.name
                      if nc.partition_id_tensor else None)
    in_names, out_names, out_avals, zero_shapes = [], [], [], []
    for alloc in nc.m.functions[0].allocations:
        if not isinstance(alloc, mybir.MemoryLocationSet):
            continue
        name = alloc.memorylocations[0].name
        if alloc.kind == "ExternalInput":
            if name != partition_name:
                in_names.append(name)
        elif alloc.kind == "ExternalOutput":
            out_names.append(name)
            shape = tuple(alloc.tensor_shape)
            dtype = mybir.dt.np(alloc.dtype)
            out_avals.append(jax.core.ShapedArray(shape, dtype))
            zero_shapes.append((shape, dtype))
    n_params = len(in_names)
    all_in_names = list(in_names) + list(out_names)
    if partition_name is not None:
        all_in_names.append(partition_name)
    donate = tuple(range(n_params, n_params + len(out_names)))

    def _body(*args):
        operands = list(args)
        if partition_name is not None:
            operands.append(bass2jax.partition_id_tensor())
        outs = bass2jax._bass_exec_p.bind(
            *operands,
            out_avals=tuple(out_avals),
            in_names=tuple(all_in_names),
            out_names=tuple(out_names),
            lowering_input_output_aliases=(),
            sim_require_finite=True,
            sim_require_nnan=True,
            nc=nc,
        )
        return tuple(outs)

    devices = jax.devices()[:NCORES]
    mesh = Mesh(np.asarray(devices), ("core",))
    nspecs = (PartitionSpec("core"),) * (n_params + len(out_names))
    sharded = jax.jit(
        shard_map(_body, mesh=mesh, in_specs=nspecs,
                  out_specs=(PartitionSpec("core"),) * len(out_names),
                  check_rep=False),
        donate_argnums=donate, keep_unused=True)
    zsh = NamedSharding(mesh, PartitionSpec("core"))
    zeros_jit = jax.jit(
        lambda: tuple(jnp.zeros((NCORES * s[0], *s[1:]), d)
                      for s, d in zero_shapes),
        out_shardings=(zsh,) * len(zero_shapes))

    def run(global_inputs):
        zs = zeros_jit()
        outs = sharded(*global_inputs, *zs)
        return [np.asarray(o) for o in outs]

    return run


def _get_runner(T):
    if T not in _RUNNER_CACHE:
        if T not in _PROGRAM_CACHE:
            _PROGRAM_CACHE[T] = _build_program(T)
        _RUNNER_CACHE[T] = _make_runner(_PROGRAM_CACHE[T])
    return _RUNNER_CACHE[T]


def _preload():
    """Heavy imports + program build + jit warmup at module import time."""
    try:
        _ensure_imports()
        import ml_dtypes
        T = _EXPECTED_T
        run = _get_runner(T)
        OFF, BLOBW, EPK = _layout(T)
        blob = np.zeros((NCORES * P, BLOBW), ml_dtypes.bfloat16)
        ep = np.zeros((P, EPK), np.uint16)
        ep[:, 2:4 * T:4] = np.arange(P, dtype=np.uint16)[:, None]
        epv = ep.view(ml_dtypes.bfloat16)
        for c in range(NCORES):
            blob[c * P:(c + 1) * P, OFF["EPU"]:OFF["EPU"] + EPK] = epv
            blob[c * P:(c + 1) * P, OFF["EPI"]:OFF["EPI"] + EPK] = epv
        run([blob])
    except Exception:
        _PROGRAM_CACHE.clear()
        _RUNNER_CACHE.clear()


# ----------------------------------------------------------------------------
# Host-side preprocessing
# ----------------------------------------------------------------------------
def _fuse_etype_weights(Wk, bk, Wv, bv, watt, wmsg):
    """Fold per-head watt/wmsg into the K/V projections. Returns [D,2D],[2D]."""
    Wk = np.asarray(Wk, np.float32); Wv = np.asarray(Wv, np.float32)
    bk = np.asarray(bk, np.float32); bv = np.asarray(bv, np.float32)
    watt = np.asarray(watt, np.float32); wmsg = np.asarray(wmsg, np.float32)
    Ak = np.empty((D, D), np.float32); Av = np.empty((D, D), np.float32)
    bak = np.empty(D, np.float32); bav = np.empty(D, np.float32)
    for h in range(H):
        sl = slice(h * DK, (h + 1) * DK)
        Ak[:, sl] = Wk[:, sl] @ watt[h]
        Av[:, sl] = Wv[:, sl] @ wmsg[h]
        bak[sl] = bk[sl] @ watt[h]
        bav[sl] = bv[sl] @ wmsg[h]
    return np.concatenate([Ak, Av], 1), np.concatenate([bak, bav])


def _q_weights(Wq, bq, mu):
    Wq = np.asarray(Wq, np.float32).copy()
    bq = np.asarray(bq, np.float32).copy()
    mu = np.asarray(mu, np.float32)
    scale = 1.0 / math.sqrt(DK)
    for h in range(H):
        sl = slice(h * DK, (h + 1) * DK)
        Wq[:, sl] *= mu[h] * scale
        bq[sl] *= mu[h] * scale
    return Wq, bq


def _build_epack(src, dst, T, EPK):
    """Per-core packed edge tiles for one etype: [P, EPK] u16 arrays with
    interleaved (src, dstloc, segdst, segid) columns per 128-edge tile."""
    order = np.argsort(dst, kind="stable")
    ds_ = dst[order].astype(np.int64)
    ss = src[order].astype(np.int64)
    core = ds_ // NSLAB
    bounds = np.searchsorted(core, np.arange(NCORES + 1))
    packs = []
    E_pad = T * P
    for c in range(NCORES):
        lo, hi = bounds[c], bounds[c + 1]
        n = hi - lo
        sloc = np.zeros(E_pad, np.int64)
        dloc = np.zeros(E_pad, np.int64)
        dkey = np.full(E_pad, 10 ** 6, np.int64)  # pad sentinel
        sloc[:n] = ss[lo:hi]
        dloc[:n] = ds_[lo:hi] - c * NSLAB
        dkey[:n] = dloc[:n]
        dk2 = dkey.reshape(T, P)
        new_seg = np.ones((T, P), bool)
        new_seg[:, 1:] = dk2[:, 1:] != dk2[:, :-1]
        segid = np.cumsum(new_seg, 1) - 1          # [T, P] local seg per edge
        segdst = (NSLAB + (np.arange(T)[:, None] + np.arange(P)[None, :])
                  % P).astype(np.int64)            # dummy rows for pad segs
        tt = np.repeat(np.arange(T), P)
        rowmap = np.where(dkey < NSLAB, dloc, segdst[tt, segid.ravel()])
        segdst[tt, segid.ravel()] = rowmap
        dloc[n:] = 0
        ep = np.zeros((P, EPK), np.uint16)
        ep[:, 0:4 * T:4] = sloc.reshape(T, P).T
        ep[:, 1:4 * T:4] = dloc.reshape(T, P).T
        ep[:, 2:4 * T:4] = segdst.T
        ep[:, 3:4 * T:4] = segid.T
        packs.append(ep)
    return packs


def kernel(feats_user, feats_item, src_ui, dst_ui, src_iu, dst_iu,
           Wk_u, bk_u, Wq_u, bq_u, Wv_u, bv_u, Wa_u, ba_u, lng_u, lnb_u, skip_u,
           Wk_i, bk_i, Wq_i, bq_i, Wv_i, bv_i, Wa_i, ba_i, lng_i, lnb_i, skip_i,
           mu_ui, watt_ui, wmsg_ui, mu_iu, watt_iu, wmsg_iu):
    _ensure_imports()
    import os
    import time as _time
    _dbg = os.environ.get("KERNEL_DEBUG_TIMING")
    _t0 = _time.time()
    import ml_dtypes
    bf = ml_dtypes.bfloat16

    feats_user = np.asarray(feats_user, np.float32)
    feats_item = np.asarray(feats_item, np.float32)
    src_ui = np.asarray(src_ui); dst_ui = np.asarray(dst_ui)
    src_iu = np.asarray(src_iu); dst_iu = np.asarray(dst_iu)

    # ---- tile count (uniform across cores/etypes) ----
    cnt = []
    for d in (dst_ui, dst_iu):
        c = np.bincount(np.asarray(d, np.int64) // NSLAB, minlength=NCORES)
        cnt.append(c.max())
    E_pad = int(np.ceil(max(cnt) / (P * 8)) * P * 8)
    T = E_pad // P
    OFF, BLOBW, EPK = _layout(T)

    # ---- fused weights ----
    kvw_ui, kvb_ui = _fuse_etype_weights(Wk_u, bk_u, Wv_u, bv_u,
                                         watt_ui, wmsg_ui)
    kvw_iu, kvb_iu = _fuse_etype_weights(Wk_i, bk_i, Wv_i, bv_i,
                                         watt_iu, wmsg_iu)
    qw_ui, qb_ui = _q_weights(Wq_i, bq_i, mu_ui)   # ui dst = item
    qw_iu, qb_iu = _q_weights(Wq_u, bq_u, mu_iu)   # iu dst = user
    al_u = 1.0 / (1.0 + math.exp(-float(np.asarray(skip_u).ravel()[0])))
    al_i = 1.0 / (1.0 + math.exp(-float(np.asarray(skip_i).ravel()[0])))
    lnw_u = np.concatenate([np.asarray(lng_u, np.float32),
                            np.asarray(lnb_u, np.float32)])
    lnw_i = np.concatenate([np.asarray(lng_i, np.float32),
                            np.asarray(lnb_i, np.float32)])
    if _dbg: print(f"[kt] weights {_time.time()-_t0:.3f}s", flush=True)

    _t1 = _time.time()
    packs_ui = _build_epack(src_ui, dst_ui, T, EPK)
    packs_iu = _build_epack(src_iu, dst_iu, T, EPK)
    if _dbg: print(f"[kt] epack {_time.time()-_t1:.3f}s", flush=True)

    # ---- assemble global blob ----
    _t2 = _time.time()
    fuT = feats_user.T.astype(bf)
    fiT = feats_item.T.astype(bf)
    blob = np.zeros((NCORES * P, BLOBW), bf)
    for c in range(NCORES):
        B = blob[c * P:(c + 1) * P]
        lo = c * NSLAB
        hi = min((c + 1) * NSLAB, N)
        B[:, OFF["FU"]:OFF["FU"] + hi - lo] = fuT[:, lo:hi]
        B[:, OFF["FI"]:OFF["FI"] + hi - lo] = fiT[:, lo:hi]
        B[:, OFF["EPU"]:OFF["EPU"] + EPK] = packs_ui[c].view(bf)
        B[:, OFF["EPI"]:OFF["EPI"] + EPK] = packs_iu[c].view(bf)
        B[:, OFF["KVW_UI"]:OFF["KVW_UI"] + 2 * D] = kvw_ui.astype(bf)
        B[:, OFF["KVW_IU"]:OFF["KVW_IU"] + 2 * D] = kvw_iu.astype(bf)
        B[:, OFF["QW_UI"]:OFF["QW_UI"] + D] = qw_ui.astype(bf)
        B[:, OFF["QW_IU"]:OFF["QW_IU"] + D] = qw_iu.astype(bf)
        B[:, OFF["WA_U"]:OFF["WA_U"] + D] = (
            al_u * np.asarray(Wa_u, np.float32)).astype(bf)
        B[:, OFF["WA_I"]:OFF["WA_I"] + D] = (
            al_i * np.asarray(Wa_i, np.float32)).astype(bf)
        B[:, OFF["IFAC_U"]:OFF["IFAC_U"] + D] = (
            (1 - al_u) * np.eye(D, dtype=np.float32)).astype(bf)
        B[:, OFF["IFAC_I"]:OFF["IFAC_I"] + D] = (
            (1 - al_i) * np.eye(D, dtype=np.float32)).astype(bf)
        v = OFF["VEC"]
        B[0, v:v + 2 * D] = kvb_ui.astype(bf)
        B[1, v:v + 2 * D] = kvb_iu.astype(bf)
        B[2, v:v + D] = qb_ui.astype(bf)
        B[3, v:v + D] = qb_iu.astype(bf)
        B[4, v:v + D] = (al_u * np.asarray(ba_u, np.float32)).astype(bf)
        B[5, v:v + D] = (al_i * np.asarray(ba_i, np.float32)).astype(bf)
        B[6, v:v + 4 * D] = lnw_u.view(np.uint16).view(bf)
        B[7, v:v + 4 * D] = lnw_i.view(np.uint16).view(bf)
    if _dbg: print(f"[kt] blob {_time.time()-_t2:.3f}s", flush=True)

    _t3 = _time.time()
    run = _get_runner(T)
    if _dbg: print(f"[kt] build {_time.time()-_t3:.3f}s", flush=True)
    _t4 = _time.time()
    outs = run([blob])
    if _dbg: print(f"[kt] run {_time.time()-_t4:.3f}s", flush=True)

    _t5 = _time.time()
    ob = outs[0].reshape(NCORES, 2, SLAB_ROWS, D)
    result = np.empty((2, N, D), np.float32)
    for c in range(NCORES):
        lo = c * NSLAB
        hi = min((c + 1) * NSLAB, N)
        result[0, lo:hi] = ob[c, 0, :hi - lo].astype(np.float32)
        result[1, lo:hi] = ob[c, 1, :hi - lo].astype(np.float32)
    if _dbg: print(f"[kt] post {_time.time()-_t5:.3f}s", flush=True)
    return result


_preload()


# revision 14
# speedup vs baseline: 1.2042x; 1.2042x over previous
"""HGT layer kernel for nn_HGTLayer_53188874994368 on 8 TRN2 NeuronCores.

Strategy (dst-partitioned edge parallel):
  - Destination nodes are partitioned into 8 slabs of 6400 rows (N padded
    to 51200).  Edges of each etype are sorted by destination and routed
    to the core owning the destination slab, so segment-softmax and
    scatter-sum are fully local per core (no cross-core reduction).
  - Node features are sharded across cores (transposed bf16 layout) and
    AllGathered on-device over NeuronLink; K/V projection tables for all
    nodes (projection weights pre-fused with the per-etype head
    transforms and mu/sqrt(dk) scaling) are computed per core on TensorE.
  - Per 128-edge tile: indirect-DMA gathers of K||V (merged table) and Q
    rows, per-head dot scores on VectorE, exp on ScalarE, one-hot segment
    matmul on TensorE, and an indirect-DMA scatter of per-segment partial
    sums.  Tiles alternate between two accumulator slabs (a segment spans
    at most 2 consecutive tiles, so each slab sees each row at most
    once); the final pass combines them, divides by the softmax
    denominator and applies the skip-gated output projection + LayerNorm
    on-device.
  - All per-core inputs travel as ONE packed bf16 blob (the axon tunnel
    charges per-array overhead), outputs as one bf16 tensor; the PJRT
    executable is jitted and warmed at import time with device-side
    donation buffers so a kernel() call pays only input upload, execute
    and output download.

Host side only sorts edges / packs index tiles (u16) and re-assembles
the output; all FLOP/bandwidth-heavy work runs on the NeuronCores.
"""

import math

import numpy as np

H = 4
DK = 32
D = 128
P = 128
N = 50000
NPAD = 51200           # 8 * 6400
NSLAB = 6400           # dst rows per core
SLAB_ROWS = NSLAB + P  # + dummy rows for padding segments
FCOLS = D + H          # 128 message cols + 4 denominator cols
LN_EPS = 1e-5
NCORES = 8
_EXPECTED_T = 608      # edge tiles per etype for the reference problem size

# ---- packed input blob layout (bf16 columns, [128, BLOBW]) ----
_SH = NSLAB + P        # feature shard cols incl. slack


def _layout(T):
    EPK = 4 * T + 32
    off = {"FU": 0, "FI": _SH}
    o = 0
    for k, w in (("EPU", EPK), ("EPI", EPK),
                 ("KVW_UI", 2 * D), ("KVW_IU", 2 * D), ("QW_UI", D),
                 ("QW_IU", D), ("WA_U", D), ("WA_I", D), ("IFAC_U", D),
                 ("IFAC_I", D), ("VEC", 4 * D)):
        off[k] = o
        o += w
    return off, o, EPK


_PROGRAM_CACHE = {}
_RUNNER_CACHE = {}


def _ensure_imports():
    import sys
    for p in ("/opt/trn_rl_repo", "/root/.axon_site/_ro/trn_rl_repo"):
        if p not in sys.path:
            sys.path.append(p)


# ----------------------------------------------------------------------------
# Device program
# ----------------------------------------------------------------------------
def _build_program(T):
    """Build the (core-uniform) Bass program for T edge-tiles per etype."""
    import concourse.bacc as bacc
    import concourse.bass as bass
    import concourse.mybir as mybir
    import concourse.tile as tile
    from concourse.bass import IndirectOffsetOnAxis
    from concourse.masks import make_identity

    bf16 = mybir.dt.bfloat16
    f32 = mybir.dt.float32
    i32 = mybir.dt.int32
    u16 = mybir.dt.uint16

    OFF, BLOBW, EPK = _layout(T)

    nc = bacc.Bacc("TRN2", target_bir_lowering=False, debug=False,
                   num_devices=NCORES, disable_frame_to_traceback=True)

    blobf = nc.dram_tensor("blobf", [P, 2 * _SH], bf16, kind="ExternalInput")
    blobi = nc.dram_tensor("blobi", [P, BLOBW], bf16, kind="ExternalInput")
    outb = nc.dram_tensor("outb", [2 * SLAB_ROWS, D], bf16,
                          kind="ExternalOutput")

    def bsl(key, w, r0=0, rows=P):
        t = blobf if key in ("FU", "FI") else blobi
        return t[r0:r0 + rows, OFF[key]:OFF[key] + w]

    # ---- internal DRAM ----
    ccin_u = nc.dram_tensor("ccin_u", [P, _SH], bf16, kind="Internal")
    ccin_i = nc.dram_tensor("ccin_i", [P, _SH], bf16, kind="Internal")
    ccout_u = nc.dram_tensor("ccout_u", [NCORES, P, _SH], bf16,
                             kind="Internal", addr_space="Shared")
    ccout_i = nc.dram_tensor("ccout_i", [NCORES, P, _SH], bf16,
                             kind="Internal", addr_space="Shared")
    KV_ui = nc.dram_tensor("KV_ui", [NPAD + P, 2 * D], bf16, kind="Internal")
    KV_iu = nc.dram_tensor("KV_iu", [NPAD + P, 2 * D], bf16, kind="Internal")
    Q_ui = nc.dram_tensor("Q_ui", [SLAB_ROWS, D], bf16, kind="Internal")
    Q_iu = nc.dram_tensor("Q_iu", [SLAB_ROWS, D], bf16, kind="Internal")
    slabs = {}
    for et in ("ui", "iu"):
        for col in ("A", "B"):
            slabs[et, col] = nc.dram_tensor(
                f"slab{col}_{et}", [SLAB_ROWS, FCOLS], f32, kind="Internal")

    with tile.TileContext(nc) as tc:
        # ---------------- phase 0: allgather features ----------------
        with tc.tile_pool(name="p0", bufs=2) as p0:
            for key, cci in (("FU", ccin_u), ("FI", ccin_i)):
                stg = p0.tile([P, _SH], bf16, tag="stg")
                nc.sync.dma_start(out=stg[:], in_=bsl(key, _SH))
                nc.sync.dma_start(out=cci[:, :], in_=stg[:])
        nc.gpsimd.collective_compute(
            "AllGather", mybir.AluOpType.bypass,
            ins=[ccin_u[:, :]], outs=[ccout_u[:, :, :]],
            replica_groups=[list(range(NCORES))])
        nc.gpsimd.collective_compute(
            "AllGather", mybir.AluOpType.bypass,
            ins=[ccin_i[:, :]], outs=[ccout_i[:, :, :]],
            replica_groups=[list(range(NCORES))])

        # ---------------- constants + weights in SBUF ----------------
        with tc.tile_pool(name="const", bufs=1) as cpool:
            zt = cpool.tile([P, 4096], f32)
            nc.vector.memset(zt[:], 0.0)
            for et in ("ui", "iu"):
                for col in ("A", "B"):
                    flat = slabs[et, col].ap().rearrange("a b -> (a b)")
                    percol = SLAB_ROWS * FCOLS // P
                    v = flat.rearrange("(p x) -> p x", p=P)
                    for c0 in range(0, percol, 4096):
                        w = min(4096, percol - c0)
                        nc.sync.dma_start(out=v[:, c0:c0 + w], in_=zt[:, :w])

            iota_i = cpool.tile([P, P], i32)
            nc.gpsimd.iota(iota_i[:], pattern=[[1, P]], base=0,
                           channel_multiplier=0)
            iota_f = cpool.tile([P, P], f32)
            nc.vector.tensor_copy(out=iota_f[:], in_=iota_i[:])
            ident_f = cpool.tile([P, P], f32)
            make_identity(nc, ident_f[:])
            ones1 = cpool.tile([1, P], bf16)
            nc.vector.memset(ones1[:], 1.0)
            ones1f = cpool.tile([1, P], f32)
            nc.vector.memset(ones1f[:], 1.0)
            epsc = cpool.tile([P, 1], f32)
            nc.vector.memset(epsc[:], LN_EPS)

            def load_mat(key, w, name):
                t = cpool.tile([P, w], bf16, name=name, tag=name)
                nc.sync.dma_start(out=t[:], in_=bsl(key, w))
                return t

            def load_vec(row, w, name):
                t = cpool.tile([1, w], bf16, name=name, tag=name)
                nc.sync.dma_start(out=t[:], in_=bsl("VEC", w, r0=row, rows=1))
                return t

            kvw_s = {"ui": load_mat("KVW_UI", 2 * D, "kvw_ui_s"),
                     "iu": load_mat("KVW_IU", 2 * D, "kvw_iu_s")}
            qw_s = {"ui": load_mat("QW_UI", D, "qw_ui_s"),
                    "iu": load_mat("QW_IU", D, "qw_iu_s")}
            wa_s = {"u": load_mat("WA_U", D, "wa_u_s"),
                    "i": load_mat("WA_I", D, "wa_i_s")}
            ifac_s = {"u": load_mat("IFAC_U", D, "ifac_u_s"),
                      "i": load_mat("IFAC_I", D, "ifac_i_s")}
            kvb_s = {"ui": load_vec(0, 2 * D, "kvb_ui_s"),
                     "iu": load_vec(1, 2 * D, "kvb_iu_s")}
            qb_s = {"ui": load_vec(2, D, "qb_ui_s"),
                    "iu": load_vec(3, D, "qb_iu_s")}
            wab_s = {"u": load_vec(4, D, "wab_u_s"),
                     "i": load_vec(5, D, "wab_i_s")}
            lnraw = {"u": load_vec(6, 4 * D, "lnraw_u"),
                     "i": load_vec(7, 4 * D, "lnraw_i")}

            # row-broadcast LN gamma||beta via K=1 matmul (f32)
            lnb = {}
            with tc.tile_pool(name="pgb", bufs=2, space="PSUM") as pgb:
                for nt in ("u", "i"):
                    ps = pgb.tile([P, 2 * D], f32, space="PSUM", tag="ps",
                                  name=f"psln_{nt}")
                    nc.tensor.matmul(out=ps[:], lhsT=ones1f[:],
                                     rhs=lnraw[nt][:].bitcast(f32),
                                     start=True, stop=True)
                    gb = cpool.tile([P, 2 * D], f32, name=f"lnbc_{nt}",
                                    tag=f"lnbc_{nt}")
                    nc.vector.tensor_copy(out=gb[:], in_=ps[:])
                    lnb[nt] = gb

            # epacks to SBUF (bf16-viewed u16 -> i32 once)
            ep_sb = {}
            for et, key in (("ui", "EPU"), ("iu", "EPI")):
                raw = cpool.tile([P, EPK], bf16, name=f"epraw_{et}",
                                 tag=f"epraw_{et}")
                nc.sync.dma_start(out=raw[:], in_=bsl(key, EPK))
                conv = cpool.tile([P, EPK], i32, name=f"ep_{et}",
                                  tag=f"ep_{et}")
                nc.vector.tensor_copy(out=conv[:], in_=raw[:].bitcast(u16))
                ep_sb[et] = conv

            # ---------------- phase 1: projection tables ----------------
            NT_B = NSLAB // P  # 50 tiles per core block
            with tc.tile_pool(name="tb", bufs=3) as tb, \
                 tc.tile_pool(name="tbp", bufs=1, space="PSUM") as tbp:
                for c in range(NCORES):
                    with tc.For_i(0, NT_B, 2) as j:
                        for uu in range(2):
                            col = (j + uu) * P
                            fu_t = tb.tile([P, P], bf16, tag=f"fu_t{uu}",
                                           name=f"fu_t{uu}")
                            nc.sync.dma_start(
                                out=fu_t[:],
                                in_=ccout_u[c, :, bass.ds(col, P)])
                            fi_t = tb.tile([P, P], bf16, tag=f"fi_t{uu}",
                                           name=f"fi_t{uu}")
                            nc.sync.dma_start(
                                out=fi_t[:],
                                in_=ccout_i[c, :, bass.ds(col, P)])
                            for et, ft in (("ui", fu_t), ("iu", fi_t)):
                                KVt = KV_ui if et == "ui" else KV_iu
                                ps = tbp.tile([P, 2 * D], f32, space="PSUM",
                                              tag=f"pkv{uu}_{et}",
                                              name=f"pkv{uu}_{et}")
                                nc.tensor.matmul(out=ps[:], lhsT=ft[:],
                                                 rhs=kvw_s[et][:], start=True,
                                                 stop=False)
                                nc.tensor.matmul(out=ps[:], lhsT=ones1[:],
                                                 rhs=kvb_s[et][:], start=False,
                                                 stop=True)
                                sb = tb.tile([P, 2 * D], bf16,
                                             tag=f"kvsb{uu}_{et}",
                                             name=f"kvsb{uu}_{et}")
                                nc.vector.tensor_copy(out=sb[:], in_=ps[:])
                                nc.sync.dma_start(
                                    out=KVt[c * NSLAB:(c + 1) * NSLAB + P, :]
                                        .rearrange("(t p) d -> t p d", p=P)
                                        [bass.ds(j + uu, 1), :, :],
                                    in_=sb[:])

                # Q tables for own slab (from own shard region of blob)
                with tc.For_i(0, NT_B, 2) as j:
                    for uu in range(2):
                        col = (j + uu) * P
                        fu_t = tb.tile([P, P], bf16, tag=f"qfu_t{uu}",
                                       name=f"qfu_t{uu}")
                        nc.sync.dma_start(
                            out=fu_t[:],
                            in_=blobf[:, OFF["FU"]:OFF["FU"] + _SH]
                                [:, bass.ds(col, P)])
                        fi_t = tb.tile([P, P], bf16, tag=f"qfi_t{uu}",
                                       name=f"qfi_t{uu}")
                        nc.sync.dma_start(
                            out=fi_t[:],
                            in_=blobf[:, OFF["FI"]:OFF["FI"] + _SH]
                                [:, bass.ds(col, P)])
                        # Q_ui: dst = item -> item feats; Q_iu: dst = user
                        for et, ft in (("ui", fi_t), ("iu", fu_t)):
                            Qt = Q_ui if et == "ui" else Q_iu
                            ps = tbp.tile([P, D], f32, space="PSUM",
                                          tag=f"pq{uu}_{et}",
                                          name=f"pq{uu}_{et}")
                            nc.tensor.matmul(out=ps[:], lhsT=ft[:],
                                             rhs=qw_s[et][:], start=True,
                                             stop=False)
                            nc.tensor.matmul(out=ps[:], lhsT=ones1[:],
                                             rhs=qb_s[et][:], start=False,
                                             stop=True)
                            sb = tb.tile([P, D], bf16, tag=f"qsb{uu}_{et}",
                                         name=f"qsb{uu}_{et}")
                            nc.vector.tensor_copy(out=sb[:], in_=ps[:])
                            nc.sync.dma_start(
                                out=Qt[0:NSLAB + P, :]
                                    .rearrange("(t p) d -> t p d", p=P)
                                    [bass.ds(j + uu, 1), :, :],
                                in_=sb[:])

            # ---------------- phase 2: edge aggregation ----------------
            UN = 8
            for et in ("ui", "iu"):
                KVt = KV_ui if et == "ui" else KV_iu
                Qt = Q_ui if et == "ui" else Q_iu
                eps = ep_sb[et]
                slA, slB = slabs[et, "A"], slabs[et, "B"]
                with tc.tile_pool(name=f"ew_{et}", bufs=2) as wp, \
                     tc.tile_pool(name=f"epp_{et}", bufs=1, space="PSUM") as pp:
                    stages = [cpool.tile([P, 4], i32, tag=f"stg_{et}{u}",
                                         name=f"stg_{et}{u}")
                              for u in range(UN)]
                    with tc.For_i(0, T, UN) as it:
                        for u in range(UN):
                            st = stages[u]
                            nc.vector.tensor_copy(
                                out=st[:],
                                in_=eps[:, bass.ds((it + u) * 4, 4)])
                            segf = wp.tile([P, 1], f32, tag=f"segf{u}",
                                           name=f"segf{u}")
                            nc.vector.tensor_copy(out=segf[:], in_=st[:, 3:4])
                            kvg = wp.tile([P, 2 * D], bf16, tag=f"kvg{u}",
                                          name=f"kvg{u}")
                            nc.gpsimd.indirect_dma_start(
                                out=kvg[:], out_offset=None, in_=KVt[:, :],
                                in_offset=IndirectOffsetOnAxis(
                                    ap=st[:, 0:1], axis=0))
                            qg = wp.tile([P, D], bf16, tag=f"qg{u}",
                                         name=f"qg{u}")
                            nc.gpsimd.indirect_dma_start(
                                out=qg[:], out_offset=None, in_=Qt[:, :],
                                in_offset=IndirectOffsetOnAxis(
                                    ap=st[:, 1:2], axis=0))
                            prod = wp.tile([P, D], f32, tag=f"prod{u}",
                                           name=f"prod{u}")
                            nc.vector.tensor_mul(out=prod[:], in0=kvg[:, 0:D],
                                                 in1=qg[:])
                            s = wp.tile([P, H], f32, tag=f"s{u}", name=f"s{u}")
                            nc.vector.tensor_reduce(
                                out=s[:],
                                in_=prod[:].rearrange("p (h d) -> p h d", h=H),
                                op=mybir.AluOpType.add,
                                axis=mybir.AxisListType.X)
                            w = wp.tile([P, H], f32, tag=f"w{u}", name=f"w{u}")
                            nc.scalar.activation(
                                out=w[:], in_=s[:],
                                func=mybir.ActivationFunctionType.Exp)
                            m = wp.tile([P, FCOLS], bf16, tag=f"m{u}",
                                        name=f"m{u}")
                            for h in range(H):
                                nc.vector.tensor_mul(
                                    out=m[:, h * DK:(h + 1) * DK],
                                    in0=kvg[:, D + h * DK:D + (h + 1) * DK],
                                    in1=w[:, h:h + 1].to_broadcast([P, DK]))
                            nc.vector.tensor_copy(out=m[:, D:FCOLS], in_=w[:])
                            S = wp.tile([P, P], bf16, tag=f"S{u}",
                                        name=f"S{u}")
                            nc.vector.tensor_tensor(
                                out=S[:],
                                in0=segf[:].to_broadcast([P, P]),
                                in1=iota_f[:], op=mybir.AluOpType.is_equal)
                            acc = pp.tile([P, FCOLS], f32, space="PSUM",
                                          tag=f"acc{u}", name=f"acc{u}")
                            nc.tensor.matmul(out=acc[:], lhsT=S[:], rhs=m[:],
                                             start=True, stop=True)
                            accs = wp.tile([P, FCOLS], f32, tag=f"accs{u}",
                                           name=f"accs{u}")
                            nc.vector.tensor_copy(out=accs[:], in_=acc[:])
                            slab = slA if u % 2 == 0 else slB
                            nc.gpsimd.indirect_dma_start(
                                out=slab[:, :],
                                out_offset=IndirectOffsetOnAxis(
                                    ap=st[:, 2:3], axis=0),
                                in_=accs[:], in_offset=None)

            # -------------- phase 3: combine + node_out + LN --------------
            for nt in ("u", "i"):
                # h_user comes from etype iu, h_item from ui
                et = "iu" if nt == "u" else "ui"
                slA, slB = slabs[et, "A"], slabs[et, "B"]
                fkey = "FU" if nt == "u" else "FI"
                nt_off = 0 if nt == "u" else SLAB_ROWS
                gbc = lnb[nt]
                with tc.tile_pool(name=f"no_{nt}", bufs=3) as op, \
                     tc.tile_pool(name=f"nop_{nt}", bufs=2, space="PSUM") as pq:
                    with tc.For_i(0, NSLAB // P, 2) as j:
                        for uu in range(2):
                            a = op.tile([P, FCOLS], f32, tag=f"fa{uu}",
                                        name=f"fa{uu}")
                            nc.sync.dma_start(
                                out=a[:],
                                in_=slA[0:NSLAB + P, :]
                                    .rearrange("(t p) d -> t p d", p=P)
                                    [bass.ds(j + uu, 1), :, :])
                            b = op.tile([P, FCOLS], f32, tag=f"fb{uu}",
                                        name=f"fb{uu}")
                            nc.sync.dma_start(
                                out=b[:],
                                in_=slB[0:NSLAB + P, :]
                                    .rearrange("(t p) d -> t p d", p=P)
                                    [bass.ds(j + uu, 1), :, :])
                            nc.vector.tensor_add(out=a[:], in0=a[:], in1=b[:])
                            den = op.tile([P, H], f32, tag=f"den{uu}",
                                          name=f"den{uu}")
                            nc.vector.tensor_scalar_add(den[:], a[:, D:FCOLS],
                                                        1e-30)
                            rec = op.tile([P, H], f32, tag=f"rec{uu}",
                                          name=f"rec{uu}")
                            nc.vector.reciprocal(out=rec[:], in_=den[:])
                            hb = op.tile([P, D], f32, tag=f"hb{uu}",
                                         name=f"hb{uu}")
                            for h in range(H):
                                nc.vector.tensor_mul(
                                    out=hb[:, h * DK:(h + 1) * DK],
                                    in0=a[:, h * DK:(h + 1) * DK],
                                    in1=rec[:, h:h + 1].to_broadcast([P, DK]))
                            hT_ps = pq.tile([P, D], f32, space="PSUM",
                                            tag=f"hT{uu}", name=f"hT{uu}")
                            nc.tensor.transpose(out=hT_ps[:], in_=hb[:],
                                                identity=ident_f[:])
                            hT = op.tile([P, D], bf16, tag=f"hTs{uu}",
                                         name=f"hTs{uu}")
                            nc.vector.tensor_copy(out=hT[:], in_=hT_ps[:])
                            fsh_t = op.tile([P, P], bf16, tag=f"fsh{uu}",
                                            name=f"fsh{uu}")
                            nc.sync.dma_start(
                                out=fsh_t[:],
                                in_=blobf[:, OFF[fkey]:OFF[fkey] + _SH]
                                    [:, bass.ds((j + uu) * P, P)])
                            ops_ = pq.tile([P, D], f32, space="PSUM",
                                           tag=f"ops{uu}", name=f"ops{uu}")
                            nc.tensor.matmul(out=ops_[:], lhsT=hT[:],
                                             rhs=wa_s[nt][:], start=True,
                                             stop=False)
                            nc.tensor.matmul(out=ops_[:], lhsT=ones1[:],
                                             rhs=wab_s[nt][:], start=False,
                                             stop=False)
                            nc.tensor.matmul(out=ops_[:], lhsT=fsh_t[:],
                                             rhs=ifac_s[nt][:], start=False,
                                             stop=True)
                            o = op.tile([P, D], f32, tag=f"o{uu}",
                                        name=f"o{uu}")
                            nc.vector.tensor_copy(out=o[:], in_=ops_[:])
                            mean = op.tile([P, 1], f32, tag=f"mean{uu}",
                                           name=f"mean{uu}")
                            nc.vector.tensor_reduce(
                                out=mean[:], in_=o[:],
                                op=mybir.AluOpType.add,
                                axis=mybir.AxisListType.X)
                            nc.vector.tensor_scalar_mul(mean[:], mean[:],
                                                        1.0 / D)
                            oc = op.tile([P, D], f32, tag=f"oc{uu}",
                                         name=f"oc{uu}")
                            nc.vector.tensor_tensor(
                                out=oc[:], in0=o[:],
                                in1=mean[:].to_broadcast([P, D]),
                                op=mybir.AluOpType.subtract)
                            sq = op.tile([P, D], f32, tag=f"sq{uu}",
                                         name=f"sq{uu}")
                            var = op.tile([P, 1], f32, tag=f"var{uu}",
                                          name=f"var{uu}")
                            nc.vector.tensor_mul(out=sq[:], in0=oc[:],
                                                 in1=oc[:])
                            nc.vector.tensor_reduce(
                                out=var[:], in_=sq[:],
                                op=mybir.AluOpType.add,
                                axis=mybir.AxisListType.X)
                            nc.vector.tensor_scalar_mul(var[:], var[:],
                                                        1.0 / D)
                            std = op.tile([P, 1], f32, tag=f"std{uu}",
                                          name=f"std{uu}")
                            nc.scalar.activation(
                                out=std[:], in_=var[:],
                                func=mybir.ActivationFunctionType.Sqrt,
                                bias=epsc[:])
                            rstd = op.tile([P, 1], f32, tag=f"rstd{uu}",
                                           name=f"rstd{uu}")
                            nc.vector.reciprocal(out=rstd[:], in_=std[:])
                            z = op.tile([P, D], f32, tag=f"z{uu}",
                                        name=f"z{uu}")
                            nc.vector.tensor_mul(
                                out=z[:], in0=oc[:],
                                in1=rstd[:].to_broadcast([P, D]))
                            zg = op.tile([P, D], f32, tag=f"zg{uu}",
                                         name=f"zg{uu}")
                            nc.vector.tensor_mul(out=zg[:], in0=z[:],
                                                 in1=gbc[:, 0:D])
                            ob = op.tile([P, D], bf16, tag=f"ob{uu}",
                                         name=f"ob{uu}")
                            nc.vector.tensor_add(out=ob[:], in0=zg[:],
                                                 in1=gbc[:, D:2 * D])
                            nc.sync.dma_start(
                                out=outb[nt_off:nt_off + SLAB_ROWS, :]
                                    .rearrange("(t p) d -> t p d", p=P)
                                    [bass.ds(j + uu, 1), :, :],
                                in_=ob[:])

    nc.compile()
    return nc


# ----------------------------------------------------------------------------
# Custom PJRT runner (jitted once; device-side donation zeros)
# ----------------------------------------------------------------------------
def _make_runner(nc):
    import jax
    import jax.numpy as jnp
    from jax.experimental.shard_map import shard_map
    from jax.sharding import Mesh, NamedSharding, PartitionSpec
    import concourse.mybir as mybir
    from concourse import bass2jax

    bass2jax.install_neuronx_cc_hook()

    partition_name = (nc.partition_id_tensor.name
                      if nc.partition_id_tensor else None)
    in_names, out_names, out_avals, zero_shapes = [], [], [], []
    for alloc in nc.m.functions[0].allocations:
        if not isinstance(alloc, mybir.MemoryLocationSet):
            continue
        name = alloc.memorylocations[0].name
        if alloc.kind == "ExternalInput":
            if name != partition_name:
                in_names.append(name)
        elif alloc.kind == "ExternalOutput":
            out_names.append(name)
            shape = tuple(alloc.tensor_shape)
            dtype = mybir.dt.np(alloc.dtype)
            out_avals.append(jax.core.ShapedArray(shape, dtype))
            zero_shapes.append((shape, dtype))
    n_params = len(in_names)
    all_in_names = list(in_names) + list(out_names)
    if partition_name is not None:
        all_in_names.append(partition_name)
    donate = tuple(range(n_params, n_params + len(out_names)))

    def _body(*args):
        operands = list(args)
        if partition_name is not None:
            operands.append(bass2jax.partition_id_tensor())
        outs = bass2jax._bass_exec_p.bind(
            *operands,
            out_avals=tuple(out_avals),
            in_names=tuple(all_in_names),
            out_names=tuple(out_names),
            lowering_input_output_aliases=(),
            sim_require_finite=True,
            sim_require_nnan=True,
            nc=nc,
        )
        return tuple(outs)

    devices = jax.devices()[:NCORES]
    mesh = Mesh(np.asarray(devices), ("core",))
    nspecs = (PartitionSpec("core"),) * (n_params + len(out_names))
    sharded = jax.jit(
        shard_map(_body, mesh=mesh, in_specs=nspecs,
                  out_specs=(PartitionSpec("core"),) * len(out_names),
                  check_rep=False),
        donate_argnums=donate, keep_unused=True)
    zsh = NamedSharding(mesh, PartitionSpec("core"))
    zeros_jit = jax.jit(
        lambda: tuple(jnp.zeros((NCORES * s[0], *s[1:]), d)
                      for s, d in zero_shapes),
        out_shardings=(zsh,) * len(zero_shapes))

    def put(x):
        return jax.device_put(x, zsh)

    def run(global_inputs):
        zs = zeros_jit()
        outs = sharded(*global_inputs, *zs)
        return [np.asarray(o) for o in outs]

    run.put = put
    return run


def _get_runner(T):
    if T not in _RUNNER_CACHE:
        if T not in _PROGRAM_CACHE:
            _PROGRAM_CACHE[T] = _build_program(T)
        _RUNNER_CACHE[T] = _make_runner(_PROGRAM_CACHE[T])
    return _RUNNER_CACHE[T]


def _preload():
    """Heavy imports + program build + jit warmup at module import time."""
    try:
        _ensure_imports()
        import ml_dtypes
        T = _EXPECTED_T
        run = _get_runner(T)
        OFF, BLOBW, EPK = _layout(T)
        blobf = np.zeros((NCORES * P, 2 * _SH), ml_dtypes.bfloat16)
        blobi = np.zeros((NCORES * P, BLOBW), ml_dtypes.bfloat16)
        ep = np.zeros((P, EPK), np.uint16)
        ep[:, 2:4 * T:4] = np.arange(P, dtype=np.uint16)[:, None]
        epv = ep.view(ml_dtypes.bfloat16)
        for c in range(NCORES):
            blobi[c * P:(c + 1) * P, OFF["EPU"]:OFF["EPU"] + EPK] = epv
            blobi[c * P:(c + 1) * P, OFF["EPI"]:OFF["EPI"] + EPK] = epv
        run([run.put(blobf), run.put(blobi)])
    except Exception:
        _PROGRAM_CACHE.clear()
        _RUNNER_CACHE.clear()


# ----------------------------------------------------------------------------
# Host-side preprocessing
# ----------------------------------------------------------------------------
def _fuse_etype_weights(Wk, bk, Wv, bv, watt, wmsg):
    """Fold per-head watt/wmsg into the K/V projections. Returns [D,2D],[2D]."""
    Wk = np.asarray(Wk, np.float32); Wv = np.asarray(Wv, np.float32)
    bk = np.asarray(bk, np.float32); bv = np.asarray(bv, np.float32)
    watt = np.asarray(watt, np.float32); wmsg = np.asarray(wmsg, np.float32)
    Ak = np.empty((D, D), np.float32); Av = np.empty((D, D), np.float32)
    bak = np.empty(D, np.float32); bav = np.empty(D, np.float32)
    for h in range(H):
        sl = slice(h * DK, (h + 1) * DK)
        Ak[:, sl] = Wk[:, sl] @ watt[h]
        Av[:, sl] = Wv[:, sl] @ wmsg[h]
        bak[sl] = bk[sl] @ watt[h]
        bav[sl] = bv[sl] @ wmsg[h]
    return np.concatenate([Ak, Av], 1), np.concatenate([bak, bav])


def _q_weights(Wq, bq, mu):
    Wq = np.asarray(Wq, np.float32).copy()
    bq = np.asarray(bq, np.float32).copy()
    mu = np.asarray(mu, np.float32)
    scale = 1.0 / math.sqrt(DK)
    for h in range(H):
        sl = slice(h * DK, (h + 1) * DK)
        Wq[:, sl] *= mu[h] * scale
        bq[sl] *= mu[h] * scale
    return Wq, bq


def _build_epack(src, dst, T, EPK):
    """Per-core packed edge tiles for one etype: [P, EPK] u16 arrays with
    interleaved (src, dstloc, segdst, segid) columns per 128-edge tile."""
    order = np.argsort(dst, kind="stable")
    ds_ = dst[order].astype(np.int64)
    ss = src[order].astype(np.int64)
    core = ds_ // NSLAB
    bounds = np.searchsorted(core, np.arange(NCORES + 1))
    packs = []
    E_pad = T * P
    for c in range(NCORES):
        lo, hi = bounds[c], bounds[c + 1]
        n = hi - lo
        sloc = np.zeros(E_pad, np.int64)
        dloc = np.zeros(E_pad, np.int64)
        dkey = np.full(E_pad, 10 ** 6, np.int64)  # pad sentinel
        sloc[:n] = ss[lo:hi]
        dloc[:n] = ds_[lo:hi] - c * NSLAB
        dkey[:n] = dloc[:n]
        dk2 = dkey.reshape(T, P)
        new_seg = np.ones((T, P), bool)
        new_seg[:, 1:] = dk2[:, 1:] != dk2[:, :-1]
        segid = np.cumsum(new_seg, 1) - 1          # [T, P] local seg per edge
        segdst = (NSLAB + (np.arange(T)[:, None] + np.arange(P)[None, :])
                  % P).astype(np.int64)            # dummy rows for pad segs
        tt = np.repeat(np.arange(T), P)
        rowmap = np.where(dkey < NSLAB, dloc, segdst[tt, segid.ravel()])
        segdst[tt, segid.ravel()] = rowmap
        dloc[n:] = 0
        ep = np.zeros((P, EPK), np.uint16)
        ep[:, 0:4 * T:4] = sloc.reshape(T, P).T
        ep[:, 1:4 * T:4] = dloc.reshape(T, P).T
        ep[:, 2:4 * T:4] = segdst.T
        ep[:, 3:4 * T:4] = segid.T
        packs.append(ep)
    return packs


def kernel(feats_user, feats_item, src_ui, dst_ui, src_iu, dst_iu,
           Wk_u, bk_u, Wq_u, bq_u, Wv_u, bv_u, Wa_u, ba_u, lng_u, lnb_u, skip_u,
           Wk_i, bk_i, Wq_i, bq_i, Wv_i, bv_i, Wa_i, ba_i, lng_i, lnb_i, skip_i,
           mu_ui, watt_ui, wmsg_ui, mu_iu, watt_iu, wmsg_iu):
    _ensure_imports()
    import os
    import time as _time
    _dbg = os.environ.get("KERNEL_DEBUG_TIMING")
    _t0 = _time.time()
    import ml_dtypes
    bf = ml_dtypes.bfloat16

    feats_user = np.asarray(feats_user, np.float32)
    feats_item = np.asarray(feats_item, np.float32)
    src_ui = np.asarray(src_ui); dst_ui = np.asarray(dst_ui)
    src_iu = np.asarray(src_iu); dst_iu = np.asarray(dst_iu)

    # ---- tile count (uniform across cores/etypes) ----
    cnt = []
    for d in (dst_ui, dst_iu):
        c = np.bincount(np.asarray(d, np.int64) // NSLAB, minlength=NCORES)
        cnt.append(c.max())
    E_pad = int(np.ceil(max(cnt) / (P * 8)) * P * 8)
    T = E_pad // P
    OFF, BLOBW, EPK = _layout(T)
    run = _get_runner(T)

    # ---- feature blob first: start its upload asynchronously ----
    _tf = _time.time()
    fuT = feats_user.T.astype(bf)
    fiT = feats_item.T.astype(bf)
    blobf = np.zeros((NCORES * P, 2 * _SH), bf)
    for c in range(NCORES):
        lo = c * NSLAB
        hi = min((c + 1) * NSLAB, N)
        B = blobf[c * P:(c + 1) * P]
        B[:, OFF["FU"]:OFF["FU"] + hi - lo] = fuT[:, lo:hi]
        B[:, OFF["FI"]:OFF["FI"] + hi - lo] = fiT[:, lo:hi]
    feats_dev = run.put(blobf)
    if _dbg: print(f"[kt] featsput {_time.time()-_tf:.3f}s", flush=True)

    # ---- fused weights ----
    kvw_ui, kvb_ui = _fuse_etype_weights(Wk_u, bk_u, Wv_u, bv_u,
                                         watt_ui, wmsg_ui)
    kvw_iu, kvb_iu = _fuse_etype_weights(Wk_i, bk_i, Wv_i, bv_i,
                                         watt_iu, wmsg_iu)
    qw_ui, qb_ui = _q_weights(Wq_i, bq_i, mu_ui)   # ui dst = item
    qw_iu, qb_iu = _q_weights(Wq_u, bq_u, mu_iu)   # iu dst = user
    al_u = 1.0 / (1.0 + math.exp(-float(np.asarray(skip_u).ravel()[0])))
    al_i = 1.0 / (1.0 + math.exp(-float(np.asarray(skip_i).ravel()[0])))
    lnw_u = np.concatenate([np.asarray(lng_u, np.float32),
                            np.asarray(lnb_u, np.float32)])
    lnw_i = np.concatenate([np.asarray(lng_i, np.float32),
                            np.asarray(lnb_i, np.float32)])
    if _dbg: print(f"[kt] weights {_time.time()-_t0:.3f}s", flush=True)

    _t1 = _time.time()
    packs_ui = _build_epack(src_ui, dst_ui, T, EPK)
    packs_iu = _build_epack(src_iu, dst_iu, T, EPK)
    if _dbg: print(f"[kt] epack {_time.time()-_t1:.3f}s", flush=True)

    # ---- assemble index/weights blob ----
    _t2 = _time.time()
    blob = np.zeros((NCORES * P, BLOBW), bf)
    for c in range(NCORES):
        B = blob[c * P:(c + 1) * P]
        B[:, OFF["EPU"]:OFF["EPU"] + EPK] = packs_ui[c].view(bf)
        B[:, OFF["EPI"]:OFF["EPI"] + EPK] = packs_iu[c].view(bf)
        B[:, OFF["KVW_UI"]:OFF["KVW_UI"] + 2 * D] = kvw_ui.astype(bf)
        B[:, OFF["KVW_IU"]:OFF["KVW_IU"] + 2 * D] = kvw_iu.astype(bf)
        B[:, OFF["QW_UI"]:OFF["QW_UI"] + D] = qw_ui.astype(bf)
        B[:, OFF["QW_IU"]:OFF["QW_IU"] + D] = qw_iu.astype(bf)
        B[:, OFF["WA_U"]:OFF["WA_U"] + D] = (
            al_u * np.asarray(Wa_u, np.float32)).astype(bf)
        B[:, OFF["WA_I"]:OFF["WA_I"] + D] = (
            al_i * np.asarray(Wa_i, np.float32)).astype(bf)
        B[:, OFF["IFAC_U"]:OFF["IFAC_U"] + D] = (
            (1 - al_u) * np.eye(D, dtype=np.float32)).astype(bf)
        B[:, OFF["IFAC_I"]:OFF["IFAC_I"] + D] = (
            (1 - al_i) * np.eye(D, dtype=np.float32)).astype(bf)
        v = OFF["VEC"]
        B[0, v:v + 2 * D] = kvb_ui.astype(bf)
        B[1, v:v + 2 * D] = kvb_iu.astype(bf)
        B[2, v:v + D] = qb_ui.astype(bf)
        B[3, v:v + D] = qb_iu.astype(bf)
        B[4, v:v + D] = (al_u * np.asarray(ba_u, np.float32)).astype(bf)
        B[5, v:v + D] = (al_i * np.asarray(ba_i, np.float32)).astype(bf)
        B[6, v:v + 4 * D] = lnw_u.view(np.uint16).view(bf)
        B[7, v:v + 4 * D] = lnw_i.view(np.uint16).view(bf)
    if _dbg: print(f"[kt] blob {_time.time()-_t2:.3f}s", flush=True)

    _t4 = _time.time()
    outs = run([feats_dev, blob])
    if _dbg: print(f"[kt] run {_time.time()-_t4:.3f}s", flush=True)

    _t5 = _time.time()
    ob = outs[0].reshape(NCORES, 2, SLAB_ROWS, D)
    result = np.empty((2, N, D), np.float32)
    for c in range(NCORES):
        lo = c * NSLAB
        hi = min((c + 1) * NSLAB, N)
        result[0, lo:hi] = ob[c, 0, :hi - lo].astype(np.float32)
        result[1, lo:hi] = ob[c, 1, :hi - lo].astype(np.float32)
    if _dbg: print(f"[kt] post {_time.time()-_t5:.3f}s", flush=True)
    return result


_preload()
